# revision 22
# baseline (speedup 1.0000x reference)
"""Any4 quantized linear (LUT dequant + GEMM + bias) on 8 Trainium2 cores.

v4: bf16 GEMM (437us/core PE floor) + software-pipelined dequant.

Column-parallel over O: OSH=512 rows per core. Host sends x transposed/cast
to bf16, packed so one DMA brings 4 k-tiles of a 512-col m-block (2KB
lines, 128 x-DMAs total); codes as a bf16 plane + 3 uint8 bit-planes; lut
repacked into 8 pair-interp tables (u_j = c*delta_j + gamma_j resolves bit0
arithmetically).

Dequant units [128 o, w i] (first quarter split 256/256/512 to cut lead-in)
run pair-interp planes (ACT x4 / Pool x4), a 7-merge copy_predicated tree
(DVE - the pole at ~8.4us/KB width), per-group affine (ACT, chunked), and a
2B DMA-transpose (scalar queue) into resident WT[t] [128, 32, 128] tiles.
Emission is software-pipelined - iteration i emits planes(i+1), merges(i),
affine+transpose(i-1) - so no engine is head-of-line blocked behind a
cross-engine dependency; effective unit wall is the DVE merge time.

GEMM sessions are [128, 512] PSUM (one bank), one per (m-block, osub).
m-block 0 keeps 4 open sessions and chases each completed k-quarter; other
blocks are visited with ragged per-osub quarter ranges as weights land,
partial sums staged to bf16 SBUF (ACT copy, DVE accumulate); after dequant
the remaining quarters run as plain sessions, copy-out fuses bias (ACT)
and re-adds staged partials (Pool).

Self-contained: hardcodes M=8192, I=4096, O=4096, G=128, n_cores=8.
"""
import sys

sys.path.insert(0, "/opt/trn_rl_repo")

import numpy as np
import ml_dtypes

import concourse.bass as bass
import concourse.mybir as mybir
import bass_rust
from concourse import tile
from concourse.bass_utils import run_bass_kernel_spmd

M, I, O, G = 8192, 4096, 4096, 128
NCORES = 8
OSH = O // NCORES          # 512 out features per core
P = 128                    # partitions
KT = I // P                # 32 k-tiles
OT = OSH // P              # 4 o-subtiles
NG = I // G                # 32 scale groups
MBLK = 512                 # m-block columns (one PSUM bank per osub)
NMB = M // MBLK            # 16 m-blocks
XK = 4                     # k-tiles per x DMA
NQ = 4                     # k-quarters (session granularity, 8 k-tiles)
KQ = KT // NQ              # k-tiles per quarter
BF = mybir.dt.bfloat16
F32 = mybir.dt.float32
U8 = mybir.dt.uint8
AF = mybir.ActivationFunctionType
OP = mybir.AluOpType

MM_US = 0.216              # one [128,512] bf16 matmul
STG_CAP = 36               # max live staged partial tiles (SBUF budget)


def _split_waits(nc, budget=1, noop_budget=1):
    """walrus rejects instructions with >1 embedded sem wait; move excess
    waits onto same-engine NoOp carriers placed directly before."""
    ctr = 0
    for fn in nc.m.functions:
        for bb in fn.blocks:
            lst = bb.instructions
            i = 0
            while i < len(lst):
                inst = lst[i]
                si = inst.sync_info
                if si is None:
                    i += 1
                    continue
                waits = list(si.on_wait or [])
                if len(waits) <= budget:
                    i += 1
                    continue
                inst.sync_info = bass_rust.SyncInfo(
                    on_wait=waits[:budget], on_update=list(si.on_update or []))
                excess = waits[budget:]
                cars = []
                for j in range(0, len(excess), noop_budget):
                    ctr += 1
                    n = mybir.InstNoOp(name=f"waitc-{ctr}", ins=[], outs=[])
                    n.engine = inst.engine
                    n.sync_info = bass_rust.SyncInfo(
                        on_wait=excess[j:j + noop_budget], on_update=[])
                    cars.append(n)
                for j, c in enumerate(cars):
                    lst.insert(i + j, c)
                i += 1 + len(cars)
    return ctr


def build():
    nc = bass.Bass()
    x_d = nc.dram_tensor("x", [KT // XK, NMB, P, XK, MBLK], BF,
                         kind="ExternalInput")
    cpl_d = nc.dram_tensor("cpl", [OSH, I], BF, kind="ExternalInput")
    predq_d = nc.dram_tensor("predq", [OSH, 3, I], U8, kind="ExternalInput")
    tbl_d = nc.dram_tensor("tbl", [OSH, 16 + 2 * NG], F32,
                           kind="ExternalInput")
    bias_d = nc.dram_tensor("bias", [P, OT], F32, kind="ExternalInput")
    yt_d = nc.dram_tensor("yt", [OSH, M], BF, kind="ExternalOutput")

    # dequant subunits (t, q, off, w): I-cols q*1024+off .. +w of o-rows
    # t*128..+128. First quarter of t0 split to shorten the lead-in chain.
    units = [(0, 0, 0, 256), (0, 0, 256, 256), (0, 0, 512, 512)]
    units += [(0, q, 0, 1024) for q in range(1, NQ)]
    for t in range(1, OT):
        units += [(t, q, 0, 1024) for q in range(NQ)]
    NU = len(units)

    def est_unit(n):
        return 12.0 * units[n][3] / 1024 + 0.3  # us, DVE planes+merges pole

    with tile.TileContext(nc) as tc:
        with (
            tc.tile_pool(name="const", bufs=1) as cpool,
            tc.tile_pool(name="tblp", bufs=1) as tblp,
            tc.tile_pool(name="cqp", bufs=5) as cqp,
            tc.tile_pool(name="pqp", bufs=6) as pqp,
            tc.tile_pool(name="dqu0", bufs=3) as dqu0,
            tc.tile_pool(name="dq", bufs=2) as dq,
            tc.tile_pool(name="xp", bufs=12) as xp,
            tc.tile_pool(name="stg", bufs=STG_CAP + 4) as stg,
            tc.tile_pool(name="yp", bufs=4) as ypool,
            tc.tile_pool(name="psy", bufs=2, space="PSUM") as psy,
        ):
            # resident transposed weights: WT[t][p, k, q] = W[t*128+q, k*128+p]
            WT = [cpool.tile([P, KT, P], BF, tag=f"wt{t}", name=f"wt{t}")
                  for t in range(OT)]

            bias_sb = cpool.tile([P, OT], F32, tag="bias", name="bias")
            nc.sync.dma_start(bias_sb[:], bias_d[:])
            tbs = []
            for t in range(OT):
                tb = tblp.tile([P, 16 + 2 * NG], F32, tag=f"tb{t}",
                               name=f"tb{t}")
                nc.sync.dma_start(tb[:], tbl_d[t * P:(t + 1) * P, :])
                tbs.append(tb)

            def emit_loads(n):
                t, q, off, w = units[n]
                osl = slice(t * P, (t + 1) * P)
                isl = slice(q * 1024 + off, q * 1024 + off + w)
                cq = cqp.tile([P, 1024], BF, tag="cq", name="cq")
                nc.sync.dma_start(cq[:, 0:w], cpl_d[osl, isl])
                pq = pqp.tile([P, 3, 1024], U8, tag="pq", name="pq")
                nc.sync.dma_start(pq[:, :, 0:w], predq_d[osl, :, isl])
                return cq, pq

            # dequant engine split: ACT does NO dequant work (it handles
            # session closes, which wait on the PE); DVE takes 6 interp
            # planes (2x bf16 tensor_scalar) + the 7-merge tree; Pool takes
            # 2 planes + the per-group affine.
            def emit_planes(n, st):
                t, q, off, w = units[n]
                cq, pq = st["loads"][n]
                tb = tbs[t]
                cqv = cq[:, 0:w]
                u = [dqu0.tile([P, 1024], BF, tag="u0", name="u0")] + \
                    [dq.tile([P, 1024], BF, tag=f"u{j}", name=f"u{j}")
                     for j in range(1, 8)]
                for j in range(8):
                    eng = nc.vector if j < 6 else nc.gpsimd
                    eng.tensor_scalar(
                        u[j][:, 0:w], cqv, tb[:, j:j + 1],
                        tb[:, 8 + j:9 + j], OP.mult, OP.add)
                st["u"][n] = u

            def emit_merges(n, st):
                t, q, off, w = units[n]
                cq, pq = st["loads"][n]
                u = st["u"][n]
                for j in range(4):
                    nc.vector.copy_predicated(
                        u[2 * j][:, 0:w], pq[:, 0, 0:w], u[2 * j + 1][:, 0:w])
                nc.vector.copy_predicated(u[0][:, 0:w], pq[:, 1, 0:w],
                                          u[2][:, 0:w])
                nc.vector.copy_predicated(u[4][:, 0:w], pq[:, 1, 0:w],
                                          u[6][:, 0:w])
                nc.vector.copy_predicated(u[0][:, 0:w], pq[:, 2, 0:w],
                                          u[4][:, 0:w])

            def emit_tail(n, st):
                t, q, off, w = units[n]
                u0 = st["u"][n][0]
                tb = tbs[t]
                wb = dq.tile([P, 1024], BF, tag="wb", name="wb")
                for gi in range(w // G):
                    g = (q * 1024 + off) // G + gi
                    gs = slice(gi * G, (gi + 1) * G)
                    nc.gpsimd.tensor_scalar(
                        wb[:, gs], u0[:, gs], tb[:, 16 + g:17 + g],
                        tb[:, 16 + NG + g:17 + NG + g], OP.mult, OP.add)
                k0 = (q * 1024 + off) // P
                nc.sync.dma_start_transpose(
                    WT[t][:, k0:k0 + w // P, :], wb[:, 0:w])

            def load_x4(k4, b):
                xk = xp.tile([P, XK, MBLK], BF, tag="xk", name="xk")
                nc.sync.dma_start(xk[:], x_d[k4, b])
                return xk

            # ---------------- scheduler state ------------------------------
            ptr = [[0] * OT for _ in range(NMB)]      # next quarter per (b,t)
            staged = [[[] for _ in range(OT)] for _ in range(NMB)]
            avail = [0] * OT                          # complete quarters per t
            sub_done = {}                             # (t,q) -> cols done
            n_staged = 0
            pe_emitted = 0.0
            deq_wall = 14.0                           # startup + first DMA
            st = {"loads": {}, "u": {}}
            pending_closes = []

            def emit_final_close(b, t, p):
                # ACT bias-fused copy-out straight to y (no staged history)
                ysb = ypool.tile([P, MBLK], BF, tag="ysb", name="ysb")
                nc.scalar.activation(
                    ysb[:], p[:], AF.Identity,
                    bias=bias_sb[:, t:t + 1], scale=1.0)
                nc.scalar.dma_start(
                    yt_d[t * P:(t + 1) * P, b * MBLK:(b + 1) * MBLK], ysb[:])

            def flush_closes():
                nonlocal pending_closes
                for b, t, p, r1 in pending_closes:
                    if r1 == NQ and not staged[b][t]:
                        emit_final_close(b, t, p)
                    else:
                        # stage (plain ACT copy); combines happen in phase B
                        s = stg.tile([P, MBLK], BF, tag="stg", name="stg")
                        nc.scalar.activation(s[:], p[:], AF.Copy)
                        staged[b][t].append(s)
                pending_closes = []

            def emit_visit(b):
                nonlocal pe_emitted, n_staged
                ranges = [(ptr[b][t], avail[t]) for t in range(OT)]
                klo = min(r0 for r0, r1 in ranges) * KQ
                khi = max(r1 for r0, r1 in ranges) * KQ
                if khi <= klo:
                    return False
                new_stage = sum(
                    1 for t in range(OT)
                    if ranges[t][1] > ranges[t][0]
                    and (ranges[t][1] < NQ or staged[b][t]))
                if n_staged + new_stage > STG_CAP:
                    return False
                n_staged += new_stage
                ps = {}
                for t in range(OT):
                    if ranges[t][1] > ranges[t][0]:
                        ps[t] = psy.tile([P, MBLK], F32, tag=f"y{t}",
                                         name=f"y{t}")
                for k4 in range(klo // XK, (khi + XK - 1) // XK):
                    xk = load_x4(k4, b)
                    for kk in range(XK):
                        k = k4 * XK + kk
                        for t, p in ps.items():
                            r0, r1 = ranges[t]
                            if r0 * KQ <= k < r1 * KQ:
                                nc.tensor.matmul(
                                    p[:], WT[t][:, k, :], xk[:, kk, :],
                                    start=(k == r0 * KQ),
                                    stop=(k == r1 * KQ - 1))
                                pe_emitted += MM_US
                for t, p in ps.items():
                    r0, r1 = ranges[t]
                    ptr[b][t] = r1
                    pending_closes.append((b, t, p, r1))
                return True

            # ---- phase A: pipelined dequant (DVE+Pool) + paced visits -----
            cursor = 0
            for i in range(3):
                st["loads"][i] = emit_loads(i)
            emit_planes(0, st)
            for n in range(NU):
                if n + 3 < NU:
                    st["loads"][n + 3] = emit_loads(n + 3)
                if n + 1 < NU:
                    emit_planes(n + 1, st)
                emit_merges(n, st)
                done_tails = []
                if n < 3:
                    emit_tail(n, st)
                    done_tails.append(n)
                elif n >= 4:
                    emit_tail(n - 1, st)
                    done_tails.append(n - 1)
                if n == NU - 1:
                    emit_tail(n, st)
                    done_tails.append(n)
                deq_wall += est_unit(n)
                for m in done_tails:
                    t, q, off, w = units[m]
                    sub_done[(t, q)] = sub_done.get((t, q), 0) + w
                    if sub_done[(t, q)] == 1024:
                        avail[t] = q + 1
                nv = 0
                while (pe_emitted < deq_wall - 10.0 and nv < 4):
                    if len(pending_closes) >= 2:
                        flush_closes()
                    tried = 0
                    while tried < NMB and not emit_visit(cursor % NMB):
                        cursor += 1
                        tried += 1
                    if tried >= NMB:
                        break
                    cursor += 1
                    nv += 1
                flush_closes()
            flush_closes()

            # ---- phase B: finish remaining quarters, combine stages -------
            for b in range(NMB):
                ps = {}
                for t in range(OT):
                    if ptr[b][t] < NQ:
                        ps[t] = psy.tile([P, MBLK], F32, tag=f"y{t}",
                                         name=f"y{t}")
                if ps:
                    klo = min(ptr[b][t] for t in range(OT) if ptr[b][t] < NQ)
                    klo *= KQ
                    for k4 in range(klo // XK, KT // XK):
                        xk = load_x4(k4, b)
                        for kk in range(XK):
                            k = k4 * XK + kk
                            for t, p in ps.items():
                                r0 = ptr[b][t]
                                if k >= r0 * KQ:
                                    nc.tensor.matmul(
                                        p[:], WT[t][:, k, :], xk[:, kk, :],
                                        start=(k == r0 * KQ),
                                        stop=(k == KT - 1))
                for t in range(OT):
                    slist = staged[b][t]
                    if not slist and t not in ps:
                        continue
                    if t in ps and not slist:
                        emit_final_close(b, t, ps[t])
                        continue
                    if t in ps:
                        tmp = ypool.tile([P, MBLK], BF, tag="tmp", name="tmp")
                        nc.scalar.activation(tmp[:], ps[t][:], AF.Copy)
                        slist = slist + [tmp]
                    # combine stages on DVE (idle post-dequant), bias last
                    acc = slist[0]
                    for s in slist[1:]:
                        nxt = ypool.tile([P, MBLK], BF, tag="acc",
                                         name="acc")
                        nc.vector.tensor_tensor(nxt[:], acc[:], s[:], OP.add)
                        acc = nxt
                    ysb = ypool.tile([P, MBLK], BF, tag="ysb", name="ysb")
                    nc.vector.tensor_scalar_add(
                        ysb[:], acc[:], bias_sb[:, t:t + 1])
                    nc.scalar.dma_start(
                        yt_d[t * P:(t + 1) * P, b * MBLK:(b + 1) * MBLK],
                        ysb[:])

    _split_waits(nc)
    return nc


def _install_ntff_shim():
    """antenv lacks axon_hooks in this image; recreate the NTFF profile hook
    via ctypes into libaxon_pjrt.so (same mechanism as trn_agent_boot)."""
    import types, contextlib, ctypes, os as _os
    if "antenv.axon_hooks" in sys.modules:
        return
    mod = types.ModuleType("antenv.axon_hooks")
    holder = {}
    mod.set_axon_ntff_profile_hook = lambda h: holder.__setitem__("h", h)
    mod.get_axon_ntff_profile_hook = lambda: holder.get("h")
    sys.modules["antenv.axon_hooks"] = mod
    try:
        import antenv
        antenv.axon_hooks = mod
    except ImportError:
        pass
    so_path = "/opt/axon/libaxon_pjrt.so"
    if not _os.path.exists(so_path):
        return
    lib = ctypes.CDLL(so_path)
    if not hasattr(lib, "axon_start_nrt_profile"):
        return
    lib.axon_start_nrt_profile.argtypes = [
        ctypes.POINTER(ctypes.c_int64), ctypes.c_size_t]
    lib.axon_start_nrt_profile.restype = ctypes.c_int64
    lib.axon_stop_nrt_profile.argtypes = [ctypes.c_char_p]
    lib.axon_stop_nrt_profile.restype = ctypes.c_int64

    @contextlib.contextmanager
    def _hook(output_dir, device_ids):
        import jax
        jax.devices()
        if device_ids:
            ids = (ctypes.c_int64 * len(device_ids))(*device_ids)
            rc = lib.axon_start_nrt_profile(ids, len(device_ids))
        else:
            rc = lib.axon_start_nrt_profile(None, 0)
        if rc != 0:
            raise RuntimeError(f"axon_start_nrt_profile rc={rc}")
        try:
            yield
        finally:
            n = lib.axon_stop_nrt_profile(str(output_dir).encode())
            print(f"ntff profile: {n} file(s) written to {output_dir}")

    mod.set_axon_ntff_profile_hook(_hook)


_NC_CACHE = None


def _get_nc():
    global _NC_CACHE
    if _NC_CACHE is None:
        _NC_CACHE = build()
    return _NC_CACHE


def _make_in_maps(input, weight, lut, scales_and_zeros, bias):
    bf16 = ml_dtypes.bfloat16
    x = np.asarray(input, dtype=np.float32)
    xT = np.ascontiguousarray(x.T).astype(bf16)        # [I, M]
    # pack [I, M] -> [KT//XK, NMB, P, XK, MBLK]: 4 k-tiles per 2KB DMA line
    x_p = np.ascontiguousarray(
        xT.reshape(KT // XK, XK, P, NMB, MBLK).transpose(0, 3, 2, 1, 4))

    codes = np.asarray(weight, dtype=np.int32)
    lut = np.asarray(lut, dtype=np.float32)
    sz = np.asarray(scales_and_zeros, dtype=np.float32)
    bias = np.asarray(bias, dtype=np.float32)
    scaleT = np.ascontiguousarray(sz[..., 0].T)        # [O, I//G]
    zeroT = np.ascontiguousarray(sz[..., 1].T)

    # pair interpolation tables: u_j = c*delta_j + gamma_j (j = c>>1)
    base = lut[:, 0::2]
    dtab = lut[:, 1::2] - base
    gtab = base - dtab * (2.0 * np.arange(8, dtype=np.float32))

    in_maps = []
    for c in range(NCORES):
        osl = slice(c * OSH, (c + 1) * OSH)
        cs = codes[osl]
        predq = np.empty((OSH, 3, I), dtype=np.uint8)
        predq[:, 0, :] = (cs >> 1) & 1
        predq[:, 1, :] = (cs >> 2) & 1
        predq[:, 2, :] = (cs >> 3) & 1
        tblv = np.concatenate([dtab[osl], gtab[osl],
                               scaleT[osl], zeroT[osl]],
                              axis=1).astype(np.float32)
        in_maps.append({
            "x": x_p,
            "cpl": np.ascontiguousarray(cs).astype(bf16),
            "predq": predq,
            "tbl": np.ascontiguousarray(tblv),
            "bias": np.ascontiguousarray(bias[osl].reshape(OT, P).T),
        })
    return in_maps


def run(input, weight, lut, scales_and_zeros, bias, trace=False, tmpdir=None):
    if trace:
        _install_ntff_shim()
        import concourse.bass_utils as _bu
        _bu.upload_artifacts = lambda d: d  # zero-egress container
    nc = _get_nc()
    in_maps = _make_in_maps(input, weight, lut, scales_and_zeros, bias)
    res = run_bass_kernel_spmd(
        nc, in_maps, list(range(NCORES)), trace=trace, tmpdir=tmpdir)
    y = np.empty((M, O), dtype=np.float32)
    for c in range(NCORES):
        yt = np.asarray(res.results[c]["yt"])  # [OSH, M] bf16
        y[:, c * OSH:(c + 1) * OSH] = yt.astype(np.float32).T
    return y, res


def kernel(input, weight, lut, scales_and_zeros, bias):
    orig_shape = np.asarray(input).shape
    y, _ = run(input, weight, lut, scales_and_zeros, bias, trace=False)
    return y.reshape(*orig_shape[:-1], O)


# revision 23
# speedup vs baseline: 1.1094x; 1.1094x over previous
"""Any4 quantized linear (LUT dequant + GEMM + bias) on 8 Trainium2 cores.

v4: bf16 GEMM (437us/core PE floor) + software-pipelined dequant.

Column-parallel over O: OSH=512 rows per core. Host sends x transposed/cast
to bf16, packed so one DMA brings 4 k-tiles of a 512-col m-block (2KB
lines, 128 x-DMAs total); codes as a bf16 plane + 3 uint8 bit-planes; lut
repacked into 8 pair-interp tables (u_j = c*delta_j + gamma_j resolves bit0
arithmetically).

Dequant units [128 o, w i] (first quarter split 256/256/512 to cut lead-in)
run pair-interp planes (ACT x4 / Pool x4), a 7-merge copy_predicated tree
(DVE - the pole at ~8.4us/KB width), per-group affine (ACT, chunked), and a
2B DMA-transpose (scalar queue) into resident WT[t] [128, 32, 128] tiles.
Emission is software-pipelined - iteration i emits planes(i+1), merges(i),
affine+transpose(i-1) - so no engine is head-of-line blocked behind a
cross-engine dependency; effective unit wall is the DVE merge time.

GEMM sessions are [128, 512] PSUM (one bank), one per (m-block, osub).
m-block 0 keeps 4 open sessions and chases each completed k-quarter; other
blocks are visited with ragged per-osub quarter ranges as weights land,
partial sums staged to bf16 SBUF (ACT copy, DVE accumulate); after dequant
the remaining quarters run as plain sessions, copy-out fuses bias (ACT)
and re-adds staged partials (Pool).

Self-contained: hardcodes M=8192, I=4096, O=4096, G=128, n_cores=8.
"""
import sys

sys.path.insert(0, "/opt/trn_rl_repo")

import numpy as np
import ml_dtypes

import concourse.bass as bass
import concourse.mybir as mybir
import bass_rust
from concourse import tile
from concourse.bass_utils import run_bass_kernel_spmd

M, I, O, G = 8192, 4096, 4096, 128
NCORES = 8
OSH = O // NCORES          # 512 out features per core
P = 128                    # partitions
KT = I // P                # 32 k-tiles
OT = OSH // P              # 4 o-subtiles
NG = I // G                # 32 scale groups
MBLK = 512                 # m-block columns (one PSUM bank per osub)
NMB = M // MBLK            # 16 m-blocks
XK = 4                     # k-tiles per x DMA
NQ = 4                     # k-quarters (session granularity, 8 k-tiles)
KQ = KT // NQ              # k-tiles per quarter
BF = mybir.dt.bfloat16
F32 = mybir.dt.float32
U8 = mybir.dt.uint8
AF = mybir.ActivationFunctionType
OP = mybir.AluOpType

MM_US = 0.216              # one [128,512] bf16 matmul
STG_CAP = 36               # max live staged partial tiles (SBUF budget)


def _split_waits(nc, budget=1, noop_budget=1):
    """walrus rejects instructions with >1 embedded sem wait; move excess
    waits onto same-engine NoOp carriers placed directly before."""
    ctr = 0
    for fn in nc.m.functions:
        for bb in fn.blocks:
            lst = bb.instructions
            i = 0
            while i < len(lst):
                inst = lst[i]
                si = inst.sync_info
                if si is None:
                    i += 1
                    continue
                waits = list(si.on_wait or [])
                if len(waits) <= budget:
                    i += 1
                    continue
                inst.sync_info = bass_rust.SyncInfo(
                    on_wait=waits[:budget], on_update=list(si.on_update or []))
                excess = waits[budget:]
                cars = []
                for j in range(0, len(excess), noop_budget):
                    ctr += 1
                    n = mybir.InstNoOp(name=f"waitc-{ctr}", ins=[], outs=[])
                    n.engine = inst.engine
                    n.sync_info = bass_rust.SyncInfo(
                        on_wait=excess[j:j + noop_budget], on_update=[])
                    cars.append(n)
                for j, c in enumerate(cars):
                    lst.insert(i + j, c)
                i += 1 + len(cars)
    return ctr


def build():
    nc = bass.Bass()
    x_d = nc.dram_tensor("x", [KT // XK, NMB, P, XK, MBLK], BF,
                         kind="ExternalInput")
    cpl_d = nc.dram_tensor("cpl", [OSH, I], BF, kind="ExternalInput")
    predq_d = nc.dram_tensor("predq", [OSH, 3, I], U8, kind="ExternalInput")
    tbl_d = nc.dram_tensor("tbl", [OSH, 16 + 2 * NG], F32,
                           kind="ExternalInput")
    bias_d = nc.dram_tensor("bias", [P, OT], F32, kind="ExternalInput")
    yt_d = nc.dram_tensor("yt", [OSH, M], BF, kind="ExternalOutput")

    # dequant subunits (t, q, off, w): I-cols q*1024+off .. +w of o-rows
    # t*128..+128. First quarter of t0 split to shorten the lead-in chain.
    units = [(0, 0, 0, 256), (0, 0, 256, 256), (0, 0, 512, 512)]
    units += [(t, 0, 0, 1024) for t in range(1, OT)]
    for q in range(1, NQ):
        units += [(t, q, 0, 1024) for t in range(OT)]
    NU = len(units)

    def est_unit(n):
        return 12.0 * units[n][3] / 1024 + 0.3  # us, DVE planes+merges pole

    with tile.TileContext(nc) as tc:
        with (
            tc.tile_pool(name="const", bufs=1) as cpool,
            tc.tile_pool(name="tblp", bufs=1) as tblp,
            tc.tile_pool(name="cqp", bufs=5) as cqp,
            tc.tile_pool(name="pqp", bufs=6) as pqp,
            tc.tile_pool(name="dqu0", bufs=3) as dqu0,
            tc.tile_pool(name="dq", bufs=2) as dq,
            tc.tile_pool(name="xp", bufs=12) as xp,
            tc.tile_pool(name="stg", bufs=STG_CAP + 4) as stg,
            tc.tile_pool(name="yp", bufs=4) as ypool,
            tc.tile_pool(name="psy", bufs=2, space="PSUM") as psy,
        ):
            # resident transposed weights: WT[t][p, k, q] = W[t*128+q, k*128+p]
            WT = [cpool.tile([P, KT, P], BF, tag=f"wt{t}", name=f"wt{t}")
                  for t in range(OT)]

            bias_sb = cpool.tile([P, OT], F32, tag="bias", name="bias")
            nc.sync.dma_start(bias_sb[:], bias_d[:])
            tbs = []
            for t in range(OT):
                tb = tblp.tile([P, 16 + 2 * NG], F32, tag=f"tb{t}",
                               name=f"tb{t}")
                nc.sync.dma_start(tb[:], tbl_d[t * P:(t + 1) * P, :])
                tbs.append(tb)

            def emit_loads(n):
                t, q, off, w = units[n]
                osl = slice(t * P, (t + 1) * P)
                isl = slice(q * 1024 + off, q * 1024 + off + w)
                cq = cqp.tile([P, 1024], BF, tag="cq", name="cq")
                nc.sync.dma_start(cq[:, 0:w], cpl_d[osl, isl])
                pq = pqp.tile([P, 3, 1024], U8, tag="pq", name="pq")
                nc.sync.dma_start(pq[:, :, 0:w], predq_d[osl, :, isl])
                return cq, pq

            # dequant engine split: ACT does NO dequant work (it handles
            # session closes, which wait on the PE); DVE takes 6 interp
            # planes (2x bf16 tensor_scalar) + the 7-merge tree; Pool takes
            # 2 planes + the per-group affine.
            def emit_planes(n, st):
                t, q, off, w = units[n]
                cq, pq = st["loads"][n]
                tb = tbs[t]
                cqv = cq[:, 0:w]
                u = [dqu0.tile([P, 1024], BF, tag="u0", name="u0")] + \
                    [dq.tile([P, 1024], BF, tag=f"u{j}", name=f"u{j}")
                     for j in range(1, 8)]
                for j in range(8):
                    eng = nc.vector if j < 6 else nc.gpsimd
                    eng.tensor_scalar(
                        u[j][:, 0:w], cqv, tb[:, j:j + 1],
                        tb[:, 8 + j:9 + j], OP.mult, OP.add)
                st["u"][n] = u

            def emit_merges(n, st):
                t, q, off, w = units[n]
                cq, pq = st["loads"][n]
                u = st["u"][n]
                for j in range(4):
                    nc.vector.copy_predicated(
                        u[2 * j][:, 0:w], pq[:, 0, 0:w], u[2 * j + 1][:, 0:w])
                nc.vector.copy_predicated(u[0][:, 0:w], pq[:, 1, 0:w],
                                          u[2][:, 0:w])
                nc.vector.copy_predicated(u[4][:, 0:w], pq[:, 1, 0:w],
                                          u[6][:, 0:w])
                nc.vector.copy_predicated(u[0][:, 0:w], pq[:, 2, 0:w],
                                          u[4][:, 0:w])

            def emit_tail(n, st):
                t, q, off, w = units[n]
                u0 = st["u"][n][0]
                tb = tbs[t]
                wb = dq.tile([P, 1024], BF, tag="wb", name="wb")
                for gi in range(w // G):
                    g = (q * 1024 + off) // G + gi
                    gs = slice(gi * G, (gi + 1) * G)
                    nc.gpsimd.tensor_scalar(
                        wb[:, gs], u0[:, gs], tb[:, 16 + g:17 + g],
                        tb[:, 16 + NG + g:17 + NG + g], OP.mult, OP.add)
                k0 = (q * 1024 + off) // P
                nc.scalar.dma_start_transpose(
                    WT[t][:, k0:k0 + w // P, :], wb[:, 0:w])

            def load_x4(k4, b):
                xk = xp.tile([P, XK, MBLK], BF, tag="xk", name="xk")
                nc.sync.dma_start(xk[:], x_d[k4, b])
                return xk

            # ---------------- scheduler state ------------------------------
            ptr = [[0] * OT for _ in range(NMB)]      # next quarter per (b,t)
            staged = [[[] for _ in range(OT)] for _ in range(NMB)]
            avail = [0] * OT                          # complete quarters per t
            sub_done = {}                             # (t,q) -> cols done
            n_staged = 0
            pe_emitted = 0.0
            deq_wall = 14.0                           # startup + first DMA
            st = {"loads": {}, "u": {}}
            pending_closes = []

            def emit_final_close(b, t, p):
                # ACT bias-fused copy-out straight to y (no staged history)
                ysb = ypool.tile([P, MBLK], BF, tag="ysb", name="ysb")
                nc.scalar.activation(
                    ysb[:], p[:], AF.Identity,
                    bias=bias_sb[:, t:t + 1], scale=1.0)
                nc.scalar.dma_start(
                    yt_d[t * P:(t + 1) * P, b * MBLK:(b + 1) * MBLK], ysb[:])

            def flush_closes():
                nonlocal pending_closes
                for b, t, p, r1 in pending_closes:
                    if r1 == NQ and not staged[b][t]:
                        emit_final_close(b, t, p)
                    else:
                        # stage (plain ACT copy); combines happen in phase B
                        s = stg.tile([P, MBLK], BF, tag="stg", name="stg")
                        nc.scalar.activation(s[:], p[:], AF.Copy)
                        staged[b][t].append(s)
                pending_closes = []

            def emit_visit(b):
                nonlocal pe_emitted, n_staged
                ranges = [(ptr[b][t], avail[t]) for t in range(OT)]
                klo = min(r0 for r0, r1 in ranges) * KQ
                khi = max(r1 for r0, r1 in ranges) * KQ
                if khi <= klo:
                    return False
                new_stage = sum(
                    1 for t in range(OT)
                    if ranges[t][1] > ranges[t][0]
                    and (ranges[t][1] < NQ or staged[b][t]))
                if n_staged + new_stage > STG_CAP:
                    return False
                n_staged += new_stage
                ps = {}
                for t in range(OT):
                    if ranges[t][1] > ranges[t][0]:
                        ps[t] = psy.tile([P, MBLK], F32, tag=f"y{t}",
                                         name=f"y{t}")
                for k4 in range(klo // XK, (khi + XK - 1) // XK):
                    xk = load_x4(k4, b)
                    for kk in range(XK):
                        k = k4 * XK + kk
                        for t, p in ps.items():
                            r0, r1 = ranges[t]
                            if r0 * KQ <= k < r1 * KQ:
                                nc.tensor.matmul(
                                    p[:], WT[t][:, k, :], xk[:, kk, :],
                                    start=(k == r0 * KQ),
                                    stop=(k == r1 * KQ - 1))
                                pe_emitted += MM_US
                for t, p in ps.items():
                    r0, r1 = ranges[t]
                    ptr[b][t] = r1
                    pending_closes.append((b, t, p, r1))
                return True

            # ---- phase A: pipelined dequant (DVE+Pool) + paced visits -----
            cursor = 0
            for i in range(3):
                st["loads"][i] = emit_loads(i)
            emit_planes(0, st)
            for n in range(NU):
                if n + 3 < NU:
                    st["loads"][n + 3] = emit_loads(n + 3)
                if n + 1 < NU:
                    emit_planes(n + 1, st)
                emit_merges(n, st)
                done_tails = []
                if n < 3:
                    emit_tail(n, st)
                    done_tails.append(n)
                elif n >= 4:
                    emit_tail(n - 1, st)
                    done_tails.append(n - 1)
                if n == NU - 1:
                    emit_tail(n, st)
                    done_tails.append(n)
                deq_wall += est_unit(n)
                for m in done_tails:
                    t, q, off, w = units[m]
                    sub_done[(t, q)] = sub_done.get((t, q), 0) + w
                    if sub_done[(t, q)] == 1024:
                        avail[t] = q + 1
                nv = 0
                while (pe_emitted < deq_wall - 10.0 and nv < 4):
                    if len(pending_closes) >= 2:
                        flush_closes()
                    tried = 0
                    while tried < NMB and not emit_visit(cursor % NMB):
                        cursor += 1
                        tried += 1
                    if tried >= NMB:
                        break
                    cursor += 1
                    nv += 1
                flush_closes()
            flush_closes()

            # ---- phase B: finish remaining quarters, combine stages -------
            for b in range(NMB):
                ps = {}
                for t in range(OT):
                    if ptr[b][t] < NQ:
                        ps[t] = psy.tile([P, MBLK], F32, tag=f"y{t}",
                                         name=f"y{t}")
                if ps:
                    klo = min(ptr[b][t] for t in range(OT) if ptr[b][t] < NQ)
                    klo *= KQ
                    for k4 in range(klo // XK, KT // XK):
                        xk = load_x4(k4, b)
                        for kk in range(XK):
                            k = k4 * XK + kk
                            for t, p in ps.items():
                                r0 = ptr[b][t]
                                if k >= r0 * KQ:
                                    nc.tensor.matmul(
                                        p[:], WT[t][:, k, :], xk[:, kk, :],
                                        start=(k == r0 * KQ),
                                        stop=(k == KT - 1))
                for t in range(OT):
                    slist = staged[b][t]
                    if not slist and t not in ps:
                        continue
                    if t in ps and not slist:
                        emit_final_close(b, t, ps[t])
                        continue
                    if t in ps:
                        tmp = ypool.tile([P, MBLK], BF, tag="tmp", name="tmp")
                        nc.scalar.activation(tmp[:], ps[t][:], AF.Copy)
                        slist = slist + [tmp]
                    # combine stages on DVE (idle post-dequant), bias last
                    acc = slist[0]
                    for s in slist[1:]:
                        nxt = ypool.tile([P, MBLK], BF, tag="acc",
                                         name="acc")
                        nc.vector.tensor_tensor(nxt[:], acc[:], s[:], OP.add)
                        acc = nxt
                    ysb = ypool.tile([P, MBLK], BF, tag="ysb", name="ysb")
                    nc.vector.tensor_scalar_add(
                        ysb[:], acc[:], bias_sb[:, t:t + 1])
                    nc.scalar.dma_start(
                        yt_d[t * P:(t + 1) * P, b * MBLK:(b + 1) * MBLK],
                        ysb[:])

    _split_waits(nc)
    return nc


def _install_ntff_shim():
    """antenv lacks axon_hooks in this image; recreate the NTFF profile hook
    via ctypes into libaxon_pjrt.so (same mechanism as trn_agent_boot)."""
    import types, contextlib, ctypes, os as _os
    if "antenv.axon_hooks" in sys.modules:
        return
    mod = types.ModuleType("antenv.axon_hooks")
    holder = {}
    mod.set_axon_ntff_profile_hook = lambda h: holder.__setitem__("h", h)
    mod.get_axon_ntff_profile_hook = lambda: holder.get("h")
    sys.modules["antenv.axon_hooks"] = mod
    try:
        import antenv
        antenv.axon_hooks = mod
    except ImportError:
        pass
    so_path = "/opt/axon/libaxon_pjrt.so"
    if not _os.path.exists(so_path):
        return
    lib = ctypes.CDLL(so_path)
    if not hasattr(lib, "axon_start_nrt_profile"):
        return
    lib.axon_start_nrt_profile.argtypes = [
        ctypes.POINTER(ctypes.c_int64), ctypes.c_size_t]
    lib.axon_start_nrt_profile.restype = ctypes.c_int64
    lib.axon_stop_nrt_profile.argtypes = [ctypes.c_char_p]
    lib.axon_stop_nrt_profile.restype = ctypes.c_int64

    @contextlib.contextmanager
    def _hook(output_dir, device_ids):
        import jax
        jax.devices()
        if device_ids:
            ids = (ctypes.c_int64 * len(device_ids))(*device_ids)
            rc = lib.axon_start_nrt_profile(ids, len(device_ids))
        else:
            rc = lib.axon_start_nrt_profile(None, 0)
        if rc != 0:
            raise RuntimeError(f"axon_start_nrt_profile rc={rc}")
        try:
            yield
        finally:
            n = lib.axon_stop_nrt_profile(str(output_dir).encode())
            print(f"ntff profile: {n} file(s) written to {output_dir}")

    mod.set_axon_ntff_profile_hook(_hook)


_NC_CACHE = None


def _get_nc():
    global _NC_CACHE
    if _NC_CACHE is None:
        _NC_CACHE = build()
    return _NC_CACHE


def _make_in_maps(input, weight, lut, scales_and_zeros, bias):
    bf16 = ml_dtypes.bfloat16
    x = np.asarray(input, dtype=np.float32)
    xT = np.ascontiguousarray(x.T).astype(bf16)        # [I, M]
    # pack [I, M] -> [KT//XK, NMB, P, XK, MBLK]: 4 k-tiles per 2KB DMA line
    x_p = np.ascontiguousarray(
        xT.reshape(KT // XK, XK, P, NMB, MBLK).transpose(0, 3, 2, 1, 4))

    codes = np.asarray(weight, dtype=np.int32)
    lut = np.asarray(lut, dtype=np.float32)
    sz = np.asarray(scales_and_zeros, dtype=np.float32)
    bias = np.asarray(bias, dtype=np.float32)
    scaleT = np.ascontiguousarray(sz[..., 0].T)        # [O, I//G]
    zeroT = np.ascontiguousarray(sz[..., 1].T)

    # pair interpolation tables: u_j = c*delta_j + gamma_j (j = c>>1)
    base = lut[:, 0::2]
    dtab = lut[:, 1::2] - base
    gtab = base - dtab * (2.0 * np.arange(8, dtype=np.float32))

    in_maps = []
    for c in range(NCORES):
        osl = slice(c * OSH, (c + 1) * OSH)
        cs = codes[osl]
        predq = np.empty((OSH, 3, I), dtype=np.uint8)
        predq[:, 0, :] = (cs >> 1) & 1
        predq[:, 1, :] = (cs >> 2) & 1
        predq[:, 2, :] = (cs >> 3) & 1
        tblv = np.concatenate([dtab[osl], gtab[osl],
                               scaleT[osl], zeroT[osl]],
                              axis=1).astype(np.float32)
        in_maps.append({
            "x": x_p,
            "cpl": np.ascontiguousarray(cs).astype(bf16),
            "predq": predq,
            "tbl": np.ascontiguousarray(tblv),
            "bias": np.ascontiguousarray(bias[osl].reshape(OT, P).T),
        })
    return in_maps


def run(input, weight, lut, scales_and_zeros, bias, trace=False, tmpdir=None):
    if trace:
        _install_ntff_shim()
        import concourse.bass_utils as _bu
        _bu.upload_artifacts = lambda d: d  # zero-egress container
    nc = _get_nc()
    in_maps = _make_in_maps(input, weight, lut, scales_and_zeros, bias)
    res = run_bass_kernel_spmd(
        nc, in_maps, list(range(NCORES)), trace=trace, tmpdir=tmpdir)
    y = np.empty((M, O), dtype=np.float32)
    for c in range(NCORES):
        yt = np.asarray(res.results[c]["yt"])  # [OSH, M] bf16
        y[:, c * OSH:(c + 1) * OSH] = yt.astype(np.float32).T
    return y, res


def kernel(input, weight, lut, scales_and_zeros, bias):
    orig_shape = np.asarray(input).shape
    y, _ = run(input, weight, lut, scales_and_zeros, bias, trace=False)
    return y.reshape(*orig_shape[:-1], O)


# revision 24
# speedup vs baseline: 1.1739x; 1.0581x over previous
"""Any4 quantized linear (LUT dequant + GEMM + bias) on 8 Trainium2 cores.

v4: bf16 GEMM (437us/core PE floor) + software-pipelined dequant.

Column-parallel over O: OSH=512 rows per core. Host sends x transposed/cast
to bf16, packed so one DMA brings 4 k-tiles of a 512-col m-block (2KB
lines, 128 x-DMAs total); codes as a bf16 plane + 3 uint8 bit-planes; lut
repacked into 8 pair-interp tables (u_j = c*delta_j + gamma_j resolves bit0
arithmetically).

Dequant units [128 o, w i] (first quarter split 256/256/512 to cut lead-in)
run pair-interp planes (ACT x4 / Pool x4), a 7-merge copy_predicated tree
(DVE - the pole at ~8.4us/KB width), per-group affine (ACT, chunked), and a
2B DMA-transpose (scalar queue) into resident WT[t] [128, 32, 128] tiles.
Emission is software-pipelined - iteration i emits planes(i+1), merges(i),
affine+transpose(i-1) - so no engine is head-of-line blocked behind a
cross-engine dependency; effective unit wall is the DVE merge time.

GEMM sessions are [128, 512] PSUM (one bank), one per (m-block, osub).
m-block 0 keeps 4 open sessions and chases each completed k-quarter; other
blocks are visited with ragged per-osub quarter ranges as weights land,
partial sums staged to bf16 SBUF (ACT copy, DVE accumulate); after dequant
the remaining quarters run as plain sessions, copy-out fuses bias (ACT)
and re-adds staged partials (Pool).

Self-contained: hardcodes M=8192, I=4096, O=4096, G=128, n_cores=8.
"""
import sys

sys.path.insert(0, "/opt/trn_rl_repo")

import numpy as np
import ml_dtypes

import concourse.bass as bass
import concourse.mybir as mybir
import bass_rust
from concourse import tile
from concourse.bass_utils import run_bass_kernel_spmd

M, I, O, G = 8192, 4096, 4096, 128
NCORES = 8
OSH = O // NCORES          # 512 out features per core
P = 128                    # partitions
KT = I // P                # 32 k-tiles
OT = OSH // P              # 4 o-subtiles
NG = I // G                # 32 scale groups
MBLK = 512                 # m-block columns (one PSUM bank per osub)
NMB = M // MBLK            # 16 m-blocks
XK = 4                     # k-tiles per x DMA
NQ = 4                     # k-quarters (session granularity, 8 k-tiles)
KQ = KT // NQ              # k-tiles per quarter
BF = mybir.dt.bfloat16
F32 = mybir.dt.float32
U8 = mybir.dt.uint8
AF = mybir.ActivationFunctionType
OP = mybir.AluOpType

MM_US = 0.216              # one [128,512] bf16 matmul
STG_CAP = 36               # max live staged partial tiles (SBUF budget)


def _split_waits(nc, budget=1, noop_budget=1):
    """walrus rejects instructions with >1 embedded sem wait; move excess
    waits onto same-engine NoOp carriers placed directly before."""
    ctr = 0
    for fn in nc.m.functions:
        for bb in fn.blocks:
            lst = bb.instructions
            i = 0
            while i < len(lst):
                inst = lst[i]
                si = inst.sync_info
                if si is None:
                    i += 1
                    continue
                waits = list(si.on_wait or [])
                if len(waits) <= budget:
                    i += 1
                    continue
                inst.sync_info = bass_rust.SyncInfo(
                    on_wait=waits[:budget], on_update=list(si.on_update or []))
                excess = waits[budget:]
                cars = []
                for j in range(0, len(excess), noop_budget):
                    ctr += 1
                    n = mybir.InstNoOp(name=f"waitc-{ctr}", ins=[], outs=[])
                    n.engine = inst.engine
                    n.sync_info = bass_rust.SyncInfo(
                        on_wait=excess[j:j + noop_budget], on_update=[])
                    cars.append(n)
                for j, c in enumerate(cars):
                    lst.insert(i + j, c)
                i += 1 + len(cars)
    return ctr


def build():
    nc = bass.Bass()
    x_d = nc.dram_tensor("x", [KT // XK, NMB, P, XK, MBLK], BF,
                         kind="ExternalInput")
    cpl_d = nc.dram_tensor("cpl", [OSH, I], BF, kind="ExternalInput")
    predq_d = nc.dram_tensor("predq", [OSH, 3, I], U8, kind="ExternalInput")
    tbl_d = nc.dram_tensor("tbl", [OSH, 16 + 2 * NG], F32,
                           kind="ExternalInput")
    bias_d = nc.dram_tensor("bias", [P, OT], F32, kind="ExternalInput")
    yt_d = nc.dram_tensor("yt", [OSH, M], BF, kind="ExternalOutput")

    # dequant subunits (t, q, off, w): I-cols q*1024+off .. +w of o-rows
    # t*128..+128. First quarter of t0 split to shorten the lead-in chain.
    units = [(0, 0, 0, 256), (0, 0, 256, 256), (0, 0, 512, 512)]
    units += [(t, 0, 0, 1024) for t in range(1, OT)]
    for q in range(1, NQ):
        units += [(t, q, 0, 1024) for t in range(OT)]
    NU = len(units)

    def est_unit(n):
        return 10.5 * units[n][3] / 1024 + 0.3  # us, measured engine pole

    with tile.TileContext(nc) as tc:
        with (
            tc.tile_pool(name="const", bufs=1) as cpool,
            tc.tile_pool(name="tblp", bufs=1) as tblp,
            tc.tile_pool(name="cqp", bufs=5) as cqp,
            tc.tile_pool(name="pqp", bufs=6) as pqp,
            tc.tile_pool(name="dqu0", bufs=3) as dqu0,
            tc.tile_pool(name="dq", bufs=2) as dq,
            tc.tile_pool(name="xp", bufs=12) as xp,
            tc.tile_pool(name="stg", bufs=STG_CAP + 4) as stg,
            tc.tile_pool(name="yp", bufs=4) as ypool,
            tc.tile_pool(name="psy", bufs=2, space="PSUM") as psy,
        ):
            # resident transposed weights: WT[t][p, k, q] = W[t*128+q, k*128+p]
            WT = [cpool.tile([P, KT, P], BF, tag=f"wt{t}", name=f"wt{t}")
                  for t in range(OT)]

            bias_sb = cpool.tile([P, OT], F32, tag="bias", name="bias")
            nc.sync.dma_start(bias_sb[:], bias_d[:])
            tbs = []
            for t in range(OT):
                tb = tblp.tile([P, 16 + 2 * NG], F32, tag=f"tb{t}",
                               name=f"tb{t}")
                nc.sync.dma_start(tb[:], tbl_d[t * P:(t + 1) * P, :])
                tbs.append(tb)

            def emit_loads(n):
                t, q, off, w = units[n]
                osl = slice(t * P, (t + 1) * P)
                isl = slice(q * 1024 + off, q * 1024 + off + w)
                cq = cqp.tile([P, 1024], BF, tag="cq", name="cq")
                nc.sync.dma_start(cq[:, 0:w], cpl_d[osl, isl])
                pq = pqp.tile([P, 3, 1024], U8, tag="pq", name="pq")
                nc.sync.dma_start(pq[:, :, 0:w], predq_d[osl, :, isl])
                return cq, pq

            # dequant engine split: ACT does NO dequant work (it handles
            # session closes, which wait on the PE); DVE takes 6 interp
            # planes (2x bf16 tensor_scalar) + the 7-merge tree; Pool takes
            # 2 planes + the per-group affine.
            def emit_planes(n, st):
                t, q, off, w = units[n]
                cq, pq = st["loads"][n]
                tb = tbs[t]
                cqv = cq[:, 0:w]
                u = [dqu0.tile([P, 1024], BF, tag="u0", name="u0")] + \
                    [dq.tile([P, 1024], BF, tag=f"u{j}", name=f"u{j}")
                     for j in range(1, 8)]
                for j in range(8):
                    if j < 4:
                        nc.scalar.activation(
                            u[j][:, 0:w], cqv, AF.Identity,
                            bias=tb[:, 8 + j:9 + j], scale=tb[:, j:j + 1])
                    else:
                        nc.gpsimd.tensor_scalar(
                            u[j][:, 0:w], cqv, tb[:, j:j + 1],
                            tb[:, 8 + j:9 + j], OP.mult, OP.add)
                st["u"][n] = u

            def emit_merges(n, st):
                t, q, off, w = units[n]
                cq, pq = st["loads"][n]
                u = st["u"][n]
                for j in range(4):
                    nc.vector.copy_predicated(
                        u[2 * j][:, 0:w], pq[:, 0, 0:w], u[2 * j + 1][:, 0:w])
                nc.vector.copy_predicated(u[0][:, 0:w], pq[:, 1, 0:w],
                                          u[2][:, 0:w])
                nc.vector.copy_predicated(u[4][:, 0:w], pq[:, 1, 0:w],
                                          u[6][:, 0:w])
                nc.vector.copy_predicated(u[0][:, 0:w], pq[:, 2, 0:w],
                                          u[4][:, 0:w])

            def emit_tail(n, st):
                t, q, off, w = units[n]
                u0 = st["u"][n][0]
                tb = tbs[t]
                wb = dq.tile([P, 1024], BF, tag="wb", name="wb")
                for gi in range(w // G):
                    g = (q * 1024 + off) // G + gi
                    gs = slice(gi * G, (gi + 1) * G)
                    nc.scalar.activation(
                        wb[:, gs], u0[:, gs], AF.Identity,
                        bias=tb[:, 16 + NG + g:17 + NG + g],
                        scale=tb[:, 16 + g:17 + g])
                k0 = (q * 1024 + off) // P
                nc.scalar.dma_start_transpose(
                    WT[t][:, k0:k0 + w // P, :], wb[:, 0:w])

            def load_x4(k4, b):
                xk = xp.tile([P, XK, MBLK], BF, tag="xk", name="xk")
                nc.sync.dma_start(xk[:], x_d[k4, b])
                return xk

            # ---------------- scheduler state ------------------------------
            ptr = [[0] * OT for _ in range(NMB)]      # next quarter per (b,t)
            staged = [[[] for _ in range(OT)] for _ in range(NMB)]
            avail = [0] * OT                          # complete quarters per t
            sub_done = {}                             # (t,q) -> cols done
            n_staged = 0
            pe_emitted = 0.0
            deq_wall = 14.0                           # startup + first DMA
            st = {"loads": {}, "u": {}}
            pending_closes = []

            def emit_final_close(b, t, p):
                # ACT bias-fused copy-out straight to y (no staged history)
                ysb = ypool.tile([P, MBLK], BF, tag="ysb", name="ysb")
                nc.scalar.activation(
                    ysb[:], p[:], AF.Identity,
                    bias=bias_sb[:, t:t + 1], scale=1.0)
                nc.scalar.dma_start(
                    yt_d[t * P:(t + 1) * P, b * MBLK:(b + 1) * MBLK], ysb[:])

            def flush_closes():
                nonlocal pending_closes
                for b, t, p, r1 in pending_closes:
                    if r1 == NQ and not staged[b][t]:
                        emit_final_close(b, t, p)
                    else:
                        # stage (plain ACT copy); combines happen in phase B
                        s = stg.tile([P, MBLK], BF, tag="stg", name="stg")
                        nc.scalar.activation(s[:], p[:], AF.Copy)
                        staged[b][t].append(s)
                pending_closes = []

            def emit_visit(b):
                nonlocal pe_emitted, n_staged
                ranges = [(ptr[b][t], avail[t]) for t in range(OT)]
                klo = min(r0 for r0, r1 in ranges) * KQ
                khi = max(r1 for r0, r1 in ranges) * KQ
                if khi <= klo:
                    return False
                new_stage = sum(
                    1 for t in range(OT)
                    if ranges[t][1] > ranges[t][0]
                    and (ranges[t][1] < NQ or staged[b][t]))
                if n_staged + new_stage > STG_CAP:
                    return False
                n_staged += new_stage
                ps = {}
                for t in range(OT):
                    if ranges[t][1] > ranges[t][0]:
                        ps[t] = psy.tile([P, MBLK], F32, tag=f"y{t}",
                                         name=f"y{t}")
                for k4 in range(klo // XK, (khi + XK - 1) // XK):
                    xk = load_x4(k4, b)
                    for kk in range(XK):
                        k = k4 * XK + kk
                        for t, p in ps.items():
                            r0, r1 = ranges[t]
                            if r0 * KQ <= k < r1 * KQ:
                                nc.tensor.matmul(
                                    p[:], WT[t][:, k, :], xk[:, kk, :],
                                    start=(k == r0 * KQ),
                                    stop=(k == r1 * KQ - 1))
                                pe_emitted += MM_US
                for t, p in ps.items():
                    r0, r1 = ranges[t]
                    ptr[b][t] = r1
                    pending_closes.append((b, t, p, r1))
                return True

            # ---- phase A: pipelined dequant (DVE+Pool) + paced visits -----
            cursor = 0
            for i in range(3):
                st["loads"][i] = emit_loads(i)
            emit_planes(0, st)
            for n in range(NU):
                if n + 3 < NU:
                    st["loads"][n + 3] = emit_loads(n + 3)
                if n + 1 < NU:
                    emit_planes(n + 1, st)
                emit_merges(n, st)
                done_tails = []
                if n < 3:
                    emit_tail(n, st)
                    done_tails.append(n)
                elif n >= 4:
                    emit_tail(n - 1, st)
                    done_tails.append(n - 1)
                if n == NU - 1:
                    emit_tail(n, st)
                    done_tails.append(n)
                deq_wall += est_unit(n)
                for m in done_tails:
                    t, q, off, w = units[m]
                    sub_done[(t, q)] = sub_done.get((t, q), 0) + w
                    if sub_done[(t, q)] == 1024:
                        avail[t] = q + 1
                nv = 0
                while (pe_emitted < 1.15 * deq_wall and nv < 2):
                    tried = 0
                    while tried < NMB and not emit_visit(cursor % NMB):
                        cursor += 1
                        tried += 1
                    if tried >= NMB:
                        break
                    cursor += 1
                    nv += 1
                flush_closes()
            flush_closes()

            # ---- phase B: finish remaining quarters, combine stages -------
            for b in range(NMB):
                ps = {}
                for t in range(OT):
                    if ptr[b][t] < NQ:
                        ps[t] = psy.tile([P, MBLK], F32, tag=f"y{t}",
                                         name=f"y{t}")
                if ps:
                    klo = min(ptr[b][t] for t in range(OT) if ptr[b][t] < NQ)
                    klo *= KQ
                    for k4 in range(klo // XK, KT // XK):
                        xk = load_x4(k4, b)
                        for kk in range(XK):
                            k = k4 * XK + kk
                            for t, p in ps.items():
                                r0 = ptr[b][t]
                                if k >= r0 * KQ:
                                    nc.tensor.matmul(
                                        p[:], WT[t][:, k, :], xk[:, kk, :],
                                        start=(k == r0 * KQ),
                                        stop=(k == KT - 1))
                for t in range(OT):
                    slist = staged[b][t]
                    if not slist and t not in ps:
                        continue
                    if t in ps and not slist:
                        emit_final_close(b, t, ps[t])
                        continue
                    if t in ps:
                        tmp = ypool.tile([P, MBLK], BF, tag="tmp", name="tmp")
                        nc.scalar.activation(tmp[:], ps[t][:], AF.Copy)
                        slist = slist + [tmp]
                    # combine stages on DVE (idle post-dequant), bias last
                    acc = slist[0]
                    for s in slist[1:]:
                        nxt = ypool.tile([P, MBLK], BF, tag="acc",
                                         name="acc")
                        nc.vector.tensor_tensor(nxt[:], acc[:], s[:], OP.add)
                        acc = nxt
                    ysb = ypool.tile([P, MBLK], BF, tag="ysb", name="ysb")
                    nc.vector.tensor_scalar_add(
                        ysb[:], acc[:], bias_sb[:, t:t + 1])
                    nc.scalar.dma_start(
                        yt_d[t * P:(t + 1) * P, b * MBLK:(b + 1) * MBLK],
                        ysb[:])

    _split_waits(nc)
    return nc


def _install_ntff_shim():
    """antenv lacks axon_hooks in this image; recreate the NTFF profile hook
    via ctypes into libaxon_pjrt.so (same mechanism as trn_agent_boot)."""
    import types, contextlib, ctypes, os as _os
    if "antenv.axon_hooks" in sys.modules:
        return
    mod = types.ModuleType("antenv.axon_hooks")
    holder = {}
    mod.set_axon_ntff_profile_hook = lambda h: holder.__setitem__("h", h)
    mod.get_axon_ntff_profile_hook = lambda: holder.get("h")
    sys.modules["antenv.axon_hooks"] = mod
    try:
        import antenv
        antenv.axon_hooks = mod
    except ImportError:
        pass
    so_path = "/opt/axon/libaxon_pjrt.so"
    if not _os.path.exists(so_path):
        return
    lib = ctypes.CDLL(so_path)
    if not hasattr(lib, "axon_start_nrt_profile"):
        return
    lib.axon_start_nrt_profile.argtypes = [
        ctypes.POINTER(ctypes.c_int64), ctypes.c_size_t]
    lib.axon_start_nrt_profile.restype = ctypes.c_int64
    lib.axon_stop_nrt_profile.argtypes = [ctypes.c_char_p]
    lib.axon_stop_nrt_profile.restype = ctypes.c_int64

    @contextlib.contextmanager
    def _hook(output_dir, device_ids):
        import jax
        jax.devices()
        if device_ids:
            ids = (ctypes.c_int64 * len(device_ids))(*device_ids)
            rc = lib.axon_start_nrt_profile(ids, len(device_ids))
        else:
            rc = lib.axon_start_nrt_profile(None, 0)
        if rc != 0:
            raise RuntimeError(f"axon_start_nrt_profile rc={rc}")
        try:
            yield
        finally:
            n = lib.axon_stop_nrt_profile(str(output_dir).encode())
            print(f"ntff profile: {n} file(s) written to {output_dir}")

    mod.set_axon_ntff_profile_hook(_hook)


_NC_CACHE = None


def _get_nc():
    global _NC_CACHE
    if _NC_CACHE is None:
        _NC_CACHE = build()
    return _NC_CACHE


def _make_in_maps(input, weight, lut, scales_and_zeros, bias):
    bf16 = ml_dtypes.bfloat16
    x = np.asarray(input, dtype=np.float32)
    xT = np.ascontiguousarray(x.T).astype(bf16)        # [I, M]
    # pack [I, M] -> [KT//XK, NMB, P, XK, MBLK]: 4 k-tiles per 2KB DMA line
    x_p = np.ascontiguousarray(
        xT.reshape(KT // XK, XK, P, NMB, MBLK).transpose(0, 3, 2, 1, 4))

    codes = np.asarray(weight, dtype=np.int32)
    lut = np.asarray(lut, dtype=np.float32)
    sz = np.asarray(scales_and_zeros, dtype=np.float32)
    bias = np.asarray(bias, dtype=np.float32)
    scaleT = np.ascontiguousarray(sz[..., 0].T)        # [O, I//G]
    zeroT = np.ascontiguousarray(sz[..., 1].T)

    # pair interpolation tables: u_j = c*delta_j + gamma_j (j = c>>1)
    base = lut[:, 0::2]
    dtab = lut[:, 1::2] - base
    gtab = base - dtab * (2.0 * np.arange(8, dtype=np.float32))

    in_maps = []
    for c in range(NCORES):
        osl = slice(c * OSH, (c + 1) * OSH)
        cs = codes[osl]
        predq = np.empty((OSH, 3, I), dtype=np.uint8)
        predq[:, 0, :] = (cs >> 1) & 1
        predq[:, 1, :] = (cs >> 2) & 1
        predq[:, 2, :] = (cs >> 3) & 1
        tblv = np.concatenate([dtab[osl], gtab[osl],
                               scaleT[osl], zeroT[osl]],
                              axis=1).astype(np.float32)
        in_maps.append({
            "x": x_p,
            "cpl": np.ascontiguousarray(cs).astype(bf16),
            "predq": predq,
            "tbl": np.ascontiguousarray(tblv),
            "bias": np.ascontiguousarray(bias[osl].reshape(OT, P).T),
        })
    return in_maps


def run(input, weight, lut, scales_and_zeros, bias, trace=False, tmpdir=None):
    if trace:
        _install_ntff_shim()
        import concourse.bass_utils as _bu
        _bu.upload_artifacts = lambda d: d  # zero-egress container
    nc = _get_nc()
    in_maps = _make_in_maps(input, weight, lut, scales_and_zeros, bias)
    res = run_bass_kernel_spmd(
        nc, in_maps, list(range(NCORES)), trace=trace, tmpdir=tmpdir)
    y = np.empty((M, O), dtype=np.float32)
    for c in range(NCORES):
        yt = np.asarray(res.results[c]["yt"])  # [OSH, M] bf16
        y[:, c * OSH:(c + 1) * OSH] = yt.astype(np.float32).T
    return y, res


def kernel(input, weight, lut, scales_and_zeros, bias):
    orig_shape = np.asarray(input).shape
    y, _ = run(input, weight, lut, scales_and_zeros, bias, trace=False)
    return y.reshape(*orig_shape[:-1], O)


# revision 28
# speedup vs baseline: 1.1903x; 1.0140x over previous
"""Any4 quantized linear (LUT dequant + GEMM + bias) on 8 Trainium2 cores.

bf16 GEMM (437us/core PE floor) + software-pipelined, dequant-overlapped
schedule. Measured: 587us HW (baseline 623-640us), rel err 3.8e-3.

Column-parallel over O: OSH=512 rows per core. Host sends x transposed/cast
to bf16, packed so one DMA brings 4 k-tiles of a 512-col m-block (2KB
lines, 128 x-DMAs total); codes as a bf16 plane + 3 uint8 bit-planes; lut
repacked into 8 pair-interp tables (u_j = c*delta_j + gamma_j resolves bit0
arithmetically).

Dequant units [128 o, w i] (first quarter split 256/256/512 to cut lead-in)
run pair-interp planes (ACT x4 / Pool x4), a 7-merge copy_predicated tree
(DVE - the pole at ~8.4us/KB width), per-group affine (ACT, chunked), and a
2B DMA-transpose (scalar queue) into resident WT[t] [128, 32, 128] tiles.
Emission is software-pipelined - iteration i emits planes(i+1), merges(i),
affine+transpose(i-1) - so no engine is head-of-line blocked behind a
cross-engine dependency; effective unit wall is the DVE merge time.

GEMM sessions are [128, 512] PSUM (one bank), one per (m-block, osub).
m-block 0 keeps 4 open sessions and chases each completed k-quarter; other
blocks are visited with ragged per-osub quarter ranges as weights land,
partial sums staged to bf16 SBUF (ACT copy, DVE accumulate); after dequant
the remaining quarters run as plain sessions, copy-out fuses bias (ACT)
and re-adds staged partials (Pool).

Self-contained: hardcodes M=8192, I=4096, O=4096, G=128, n_cores=8.
"""
import sys

sys.path.insert(0, "/opt/trn_rl_repo")

import numpy as np
import ml_dtypes

import concourse.bass as bass
import concourse.mybir as mybir
import bass_rust
from concourse import tile
from concourse.bass_utils import run_bass_kernel_spmd

M, I, O, G = 8192, 4096, 4096, 128
NCORES = 8
OSH = O // NCORES          # 512 out features per core
P = 128                    # partitions
KT = I // P                # 32 k-tiles
OT = OSH // P              # 4 o-subtiles
NG = I // G                # 32 scale groups
MBLK = 512                 # m-block columns (one PSUM bank per osub)
NMB = M // MBLK            # 16 m-blocks
XK = 4                     # k-tiles per x DMA
NQ = 4                     # k-quarters (session granularity, 8 k-tiles)
KQ = KT // NQ              # k-tiles per quarter
BF = mybir.dt.bfloat16
F32 = mybir.dt.float32
U8 = mybir.dt.uint8
AF = mybir.ActivationFunctionType
OP = mybir.AluOpType

MM_US = 0.216              # one [128,512] bf16 matmul
STG_CAP = 36               # max live staged partial tiles (SBUF budget)


def _split_waits(nc, budget=1, noop_budget=1):
    """walrus rejects instructions with >1 embedded sem wait; move excess
    waits onto same-engine NoOp carriers placed directly before."""
    ctr = 0
    for fn in nc.m.functions:
        for bb in fn.blocks:
            lst = bb.instructions
            i = 0
            while i < len(lst):
                inst = lst[i]
                si = inst.sync_info
                if si is None:
                    i += 1
                    continue
                waits = list(si.on_wait or [])
                if len(waits) <= budget:
                    i += 1
                    continue
                inst.sync_info = bass_rust.SyncInfo(
                    on_wait=waits[:budget], on_update=list(si.on_update or []))
                excess = waits[budget:]
                cars = []
                for j in range(0, len(excess), noop_budget):
                    ctr += 1
                    n = mybir.InstNoOp(name=f"waitc-{ctr}", ins=[], outs=[])
                    n.engine = inst.engine
                    n.sync_info = bass_rust.SyncInfo(
                        on_wait=excess[j:j + noop_budget], on_update=[])
                    cars.append(n)
                for j, c in enumerate(cars):
                    lst.insert(i + j, c)
                i += 1 + len(cars)
    return ctr


def build():
    nc = bass.Bass()
    x_d = nc.dram_tensor("x", [KT // XK, NMB, P, XK, MBLK], BF,
                         kind="ExternalInput")
    cpl_d = nc.dram_tensor("cpl", [OSH, I], BF, kind="ExternalInput")
    predq_d = nc.dram_tensor("predq", [OSH, 3, I], U8, kind="ExternalInput")
    tbl_d = nc.dram_tensor("tbl", [OSH, 16 + 2 * NG], F32,
                           kind="ExternalInput")
    bias_d = nc.dram_tensor("bias", [P, OT], F32, kind="ExternalInput")
    yt_d = nc.dram_tensor("yt", [OSH, M], BF, kind="ExternalOutput")

    # dequant subunits (t, q, off, w): I-cols q*1024+off .. +w of o-rows
    # t*128..+128. First quarter of t0 split to shorten the lead-in chain.
    units = [(0, 0, 0, 256), (0, 0, 256, 256), (0, 0, 512, 512)]
    units += [(t, 0, 0, 1024) for t in range(1, OT)]
    for q in range(1, NQ):
        units += [(t, q, 0, 1024) for t in range(OT)]
    NU = len(units)

    def est_unit(n):
        return 12.0 * units[n][3] / 1024 + 0.3  # us, measured engine pole

    with tile.TileContext(nc) as tc:
        with (
            tc.tile_pool(name="const", bufs=1) as cpool,
            tc.tile_pool(name="tblp", bufs=1) as tblp,
            tc.tile_pool(name="cqp", bufs=5) as cqp,
            tc.tile_pool(name="pqp", bufs=6) as pqp,
            tc.tile_pool(name="dqu0", bufs=3) as dqu0,
            tc.tile_pool(name="dq", bufs=2) as dq,
            tc.tile_pool(name="xp", bufs=12) as xp,
            tc.tile_pool(name="stg", bufs=STG_CAP + 4) as stg,
            tc.tile_pool(name="yp", bufs=4) as ypool,
            tc.tile_pool(name="psy", bufs=2, space="PSUM") as psy,
        ):
            # resident transposed weights: WT[t][p, k, q] = W[t*128+q, k*128+p]
            WT = [cpool.tile([P, KT, P], BF, tag=f"wt{t}", name=f"wt{t}")
                  for t in range(OT)]

            bias_sb = cpool.tile([P, OT], F32, tag="bias", name="bias")
            nc.sync.dma_start(bias_sb[:], bias_d[:])
            tbs = []
            for t in range(OT):
                tb = tblp.tile([P, 16 + 2 * NG], F32, tag=f"tb{t}",
                               name=f"tb{t}")
                nc.sync.dma_start(tb[:], tbl_d[t * P:(t + 1) * P, :])
                tbs.append(tb)

            def emit_loads(n):
                t, q, off, w = units[n]
                osl = slice(t * P, (t + 1) * P)
                isl = slice(q * 1024 + off, q * 1024 + off + w)
                cq = cqp.tile([P, 1024], BF, tag="cq", name="cq")
                nc.sync.dma_start(cq[:, 0:w], cpl_d[osl, isl])
                pq = pqp.tile([P, 3, 1024], U8, tag="pq", name="pq")
                nc.sync.dma_start(pq[:, :, 0:w], predq_d[osl, :, isl])
                return cq, pq

            # dequant engine split: ACT takes 4 interp planes + the
            # per-group affine; Pool takes 4 planes; DVE runs the 7-merge
            # copy_predicated tree (the pole). Session closes (ACT) are
            # deferred one pipeline iteration so their PE-completion waits
            # don't head-of-line block the dequant stream.
            def emit_planes(n, st):
                t, q, off, w = units[n]
                cq, pq = st["loads"][n]
                tb = tbs[t]
                cqv = cq[:, 0:w]
                u = [dqu0.tile([P, 1024], BF, tag="u0", name="u0")] + \
                    [dq.tile([P, 1024], BF, tag=f"u{j}", name=f"u{j}")
                     for j in range(1, 8)]
                for j in range(8):
                    if j < 4:
                        nc.scalar.activation(
                            u[j][:, 0:w], cqv, AF.Identity,
                            bias=tb[:, 8 + j:9 + j], scale=tb[:, j:j + 1])
                    else:
                        nc.gpsimd.tensor_scalar(
                            u[j][:, 0:w], cqv, tb[:, j:j + 1],
                            tb[:, 8 + j:9 + j], OP.mult, OP.add)
                st["u"][n] = u

            def emit_merges(n, st):
                t, q, off, w = units[n]
                cq, pq = st["loads"][n]
                u = st["u"][n]
                for j in range(4):
                    nc.vector.copy_predicated(
                        u[2 * j][:, 0:w], pq[:, 0, 0:w], u[2 * j + 1][:, 0:w])
                nc.vector.copy_predicated(u[0][:, 0:w], pq[:, 1, 0:w],
                                          u[2][:, 0:w])
                nc.vector.copy_predicated(u[4][:, 0:w], pq[:, 1, 0:w],
                                          u[6][:, 0:w])
                nc.vector.copy_predicated(u[0][:, 0:w], pq[:, 2, 0:w],
                                          u[4][:, 0:w])

            def emit_tail(n, st):
                t, q, off, w = units[n]
                u0 = st["u"][n][0]
                tb = tbs[t]
                wb = dq.tile([P, 1024], BF, tag="wb", name="wb")
                for gi in range(w // G):
                    g = (q * 1024 + off) // G + gi
                    gs = slice(gi * G, (gi + 1) * G)
                    nc.scalar.activation(
                        wb[:, gs], u0[:, gs], AF.Identity,
                        bias=tb[:, 16 + NG + g:17 + NG + g],
                        scale=tb[:, 16 + g:17 + g])
                k0 = (q * 1024 + off) // P
                nc.scalar.dma_start_transpose(
                    WT[t][:, k0:k0 + w // P, :], wb[:, 0:w])

            def load_x4(k4, b):
                xk = xp.tile([P, XK, MBLK], BF, tag="xk", name="xk")
                nc.sync.dma_start(xk[:], x_d[k4, b])
                return xk

            # ---------------- scheduler state ------------------------------
            ptr = [[0] * OT for _ in range(NMB)]      # next quarter per (b,t)
            staged = [[[] for _ in range(OT)] for _ in range(NMB)]
            avail = [0] * OT                          # complete quarters per t
            sub_done = {}                             # (t,q) -> cols done
            n_staged = 0
            pe_emitted = 0.0
            deq_wall = 14.0                           # startup + first DMA
            st = {"loads": {}, "u": {}}
            pending_closes = []

            def emit_final_close(b, t, p):
                # ACT bias-fused copy-out straight to y (no staged history)
                ysb = ypool.tile([P, MBLK], BF, tag="ysb", name="ysb")
                nc.scalar.activation(
                    ysb[:], p[:], AF.Identity,
                    bias=bias_sb[:, t:t + 1], scale=1.0)
                nc.scalar.dma_start(
                    yt_d[t * P:(t + 1) * P, b * MBLK:(b + 1) * MBLK], ysb[:])

            def flush_closes():
                nonlocal pending_closes
                for b, t, p, r1 in pending_closes:
                    if r1 == NQ and not staged[b][t]:
                        emit_final_close(b, t, p)
                    else:
                        # stage (plain ACT copy); combines happen in phase B
                        s = stg.tile([P, MBLK], BF, tag="stg", name="stg")
                        nc.scalar.activation(s[:], p[:], AF.Copy)
                        staged[b][t].append(s)
                pending_closes = []

            def emit_visit(b):
                nonlocal pe_emitted, n_staged
                ranges = [(ptr[b][t], avail[t]) for t in range(OT)]
                klo = min(r0 for r0, r1 in ranges) * KQ
                khi = max(r1 for r0, r1 in ranges) * KQ
                if khi <= klo:
                    return False
                new_stage = sum(
                    1 for t in range(OT)
                    if ranges[t][1] > ranges[t][0]
                    and (ranges[t][1] < NQ or staged[b][t]))
                if n_staged + new_stage > STG_CAP:
                    return False
                n_staged += new_stage
                ps = {}
                for t in range(OT):
                    if ranges[t][1] > ranges[t][0]:
                        ps[t] = psy.tile([P, MBLK], F32, tag=f"y{t}",
                                         name=f"y{t}")
                for k4 in range(klo // XK, (khi + XK - 1) // XK):
                    xk = load_x4(k4, b)
                    for kk in range(XK):
                        k = k4 * XK + kk
                        for t, p in ps.items():
                            r0, r1 = ranges[t]
                            if r0 * KQ <= k < r1 * KQ:
                                nc.tensor.matmul(
                                    p[:], WT[t][:, k, :], xk[:, kk, :],
                                    start=(k == r0 * KQ),
                                    stop=(k == r1 * KQ - 1))
                                pe_emitted += MM_US
                for t, p in ps.items():
                    r0, r1 = ranges[t]
                    ptr[b][t] = r1
                    pending_closes.append((b, t, p, r1))
                return True

            # ---- phase A: pipelined dequant (DVE+Pool) + paced visits -----
            cursor = 0
            for i in range(3):
                st["loads"][i] = emit_loads(i)
            emit_planes(0, st)
            for n in range(NU):
                if n + 3 < NU:
                    st["loads"][n + 3] = emit_loads(n + 3)
                if n + 1 < NU:
                    emit_planes(n + 1, st)
                emit_merges(n, st)
                done_tails = []
                if n < 3:
                    emit_tail(n, st)
                    done_tails.append(n)
                elif n >= 4:
                    emit_tail(n - 1, st)
                    done_tails.append(n - 1)
                if n == NU - 1:
                    emit_tail(n, st)
                    done_tails.append(n)
                deq_wall += est_unit(n)
                for m in done_tails:
                    t, q, off, w = units[m]
                    sub_done[(t, q)] = sub_done.get((t, q), 0) + w
                    if sub_done[(t, q)] == 1024:
                        avail[t] = q + 1
                nv = 0
                while (pe_emitted < 1.15 * deq_wall and nv < 2):
                    tried = 0
                    while tried < NMB and not emit_visit(cursor % NMB):
                        cursor += 1
                        tried += 1
                    if tried >= NMB:
                        break
                    cursor += 1
                    nv += 1
                flush_closes()
            flush_closes()

            # ---- phase B: finish remaining quarters, combine stages -------
            for b in range(NMB):
                ps = {}
                for t in range(OT):
                    if ptr[b][t] < NQ:
                        ps[t] = psy.tile([P, MBLK], F32, tag=f"y{t}",
                                         name=f"y{t}")
                if ps:
                    klo = min(ptr[b][t] for t in range(OT) if ptr[b][t] < NQ)
                    klo *= KQ
                    for k4 in range(klo // XK, KT // XK):
                        xk = load_x4(k4, b)
                        for kk in range(XK):
                            k = k4 * XK + kk
                            for t, p in ps.items():
                                r0 = ptr[b][t]
                                if k >= r0 * KQ:
                                    nc.tensor.matmul(
                                        p[:], WT[t][:, k, :], xk[:, kk, :],
                                        start=(k == r0 * KQ),
                                        stop=(k == KT - 1))
                for t in range(OT):
                    slist = staged[b][t]
                    if not slist and t not in ps:
                        continue
                    if t in ps and not slist:
                        emit_final_close(b, t, ps[t])
                        continue
                    if t in ps:
                        tmp = ypool.tile([P, MBLK], BF, tag="tmp", name="tmp")
                        nc.scalar.activation(tmp[:], ps[t][:], AF.Copy)
                        slist = slist + [tmp]
                    # combine stages on DVE (idle post-dequant), bias last
                    acc = slist[0]
                    for s in slist[1:]:
                        nxt = ypool.tile([P, MBLK], BF, tag="acc",
                                         name="acc")
                        nc.vector.tensor_tensor(nxt[:], acc[:], s[:], OP.add)
                        acc = nxt
                    ysb = ypool.tile([P, MBLK], BF, tag="ysb", name="ysb")
                    nc.vector.tensor_scalar_add(
                        ysb[:], acc[:], bias_sb[:, t:t + 1])
                    nc.scalar.dma_start(
                        yt_d[t * P:(t + 1) * P, b * MBLK:(b + 1) * MBLK],
                        ysb[:])

    _split_waits(nc)
    return nc


def _install_ntff_shim():
    """antenv lacks axon_hooks in this image; recreate the NTFF profile hook
    via ctypes into libaxon_pjrt.so (same mechanism as trn_agent_boot)."""
    import types, contextlib, ctypes, os as _os
    if "antenv.axon_hooks" in sys.modules:
        return
    mod = types.ModuleType("antenv.axon_hooks")
    holder = {}
    mod.set_axon_ntff_profile_hook = lambda h: holder.__setitem__("h", h)
    mod.get_axon_ntff_profile_hook = lambda: holder.get("h")
    sys.modules["antenv.axon_hooks"] = mod
    try:
        import antenv
        antenv.axon_hooks = mod
    except ImportError:
        pass
    so_path = "/opt/axon/libaxon_pjrt.so"
    if not _os.path.exists(so_path):
        return
    lib = ctypes.CDLL(so_path)
    if not hasattr(lib, "axon_start_nrt_profile"):
        return
    lib.axon_start_nrt_profile.argtypes = [
        ctypes.POINTER(ctypes.c_int64), ctypes.c_size_t]
    lib.axon_start_nrt_profile.restype = ctypes.c_int64
    lib.axon_stop_nrt_profile.argtypes = [ctypes.c_char_p]
    lib.axon_stop_nrt_profile.restype = ctypes.c_int64

    @contextlib.contextmanager
    def _hook(output_dir, device_ids):
        import jax
        jax.devices()
        if device_ids:
            ids = (ctypes.c_int64 * len(device_ids))(*device_ids)
            rc = lib.axon_start_nrt_profile(ids, len(device_ids))
        else:
            rc = lib.axon_start_nrt_profile(None, 0)
        if rc != 0:
            raise RuntimeError(f"axon_start_nrt_profile rc={rc}")
        try:
            yield
        finally:
            n = lib.axon_stop_nrt_profile(str(output_dir).encode())
            print(f"ntff profile: {n} file(s) written to {output_dir}")

    mod.set_axon_ntff_profile_hook(_hook)


_NC_CACHE = None


def _get_nc():
    global _NC_CACHE
    if _NC_CACHE is None:
        _NC_CACHE = build()
    return _NC_CACHE


def _make_in_maps(input, weight, lut, scales_and_zeros, bias):
    bf16 = ml_dtypes.bfloat16
    x = np.asarray(input, dtype=np.float32)
    xT = np.ascontiguousarray(x.T).astype(bf16)        # [I, M]
    # pack [I, M] -> [KT//XK, NMB, P, XK, MBLK]: 4 k-tiles per 2KB DMA line
    x_p = np.ascontiguousarray(
        xT.reshape(KT // XK, XK, P, NMB, MBLK).transpose(0, 3, 2, 1, 4))

    codes = np.asarray(weight, dtype=np.int32)
    lut = np.asarray(lut, dtype=np.float32)
    sz = np.asarray(scales_and_zeros, dtype=np.float32)
    bias = np.asarray(bias, dtype=np.float32)
    scaleT = np.ascontiguousarray(sz[..., 0].T)        # [O, I//G]
    zeroT = np.ascontiguousarray(sz[..., 1].T)

    # pair interpolation tables: u_j = c*delta_j + gamma_j (j = c>>1)
    base = lut[:, 0::2]
    dtab = lut[:, 1::2] - base
    gtab = base - dtab * (2.0 * np.arange(8, dtype=np.float32))

    in_maps = []
    for c in range(NCORES):
        osl = slice(c * OSH, (c + 1) * OSH)
        cs = codes[osl]
        predq = np.empty((OSH, 3, I), dtype=np.uint8)
        predq[:, 0, :] = (cs >> 1) & 1
        predq[:, 1, :] = (cs >> 2) & 1
        predq[:, 2, :] = (cs >> 3) & 1
        tblv = np.concatenate([dtab[osl], gtab[osl],
                               scaleT[osl], zeroT[osl]],
                              axis=1).astype(np.float32)
        in_maps.append({
            "x": x_p,
            "cpl": np.ascontiguousarray(cs).astype(bf16),
            "predq": predq,
            "tbl": np.ascontiguousarray(tblv),
            "bias": np.ascontiguousarray(bias[osl].reshape(OT, P).T),
        })
    return in_maps


def run(input, weight, lut, scales_and_zeros, bias, trace=False, tmpdir=None):
    if trace:
        _install_ntff_shim()
        import concourse.bass_utils as _bu
        _bu.upload_artifacts = lambda d: d  # zero-egress container
    nc = _get_nc()
    in_maps = _make_in_maps(input, weight, lut, scales_and_zeros, bias)
    res = run_bass_kernel_spmd(
        nc, in_maps, list(range(NCORES)), trace=trace, tmpdir=tmpdir)
    y = np.empty((M, O), dtype=np.float32)
    for c in range(NCORES):
        yt = np.asarray(res.results[c]["yt"])  # [OSH, M] bf16
        y[:, c * OSH:(c + 1) * OSH] = yt.astype(np.float32).T
    return y, res


def kernel(input, weight, lut, scales_and_zeros, bias):
    orig_shape = np.asarray(input).shape
    y, _ = run(input, weight, lut, scales_and_zeros, bias, trace=False)
    return y.reshape(*orig_shape[:-1], O)
